# revision 6
# baseline (speedup 1.0000x reference)
"""Trainium2 Bass kernel for nn_MNIST_CNN (SCINOL effective-parameter CNN).

Data-parallel over batch: 2048 images -> 8 cores x 256. SCINOL M-updates
need full-batch statistics -> 3 tiny AllReduces (1, 64, 1024 floats).

Per-core pipeline (matmuls bf16, f32 accumulate):
  stat1(|x|) -> AR -> w1_eff -> conv1 (im2col K=30, M=128 packs 2 output
  rows; im2col built in DRAM once, streamed per chunk) -> pool1 -> h1
  [64c x (n,12,12)] bf16, duplicated on partitions 64..127 shifted +1 in x
  so conv2 runs K=128 (two taps per matmul) -> stat2 -> AR -> w2_eff ->
  conv2 (strided-view rhs, 15 matmuls/chunk) -> pool2 -> h2 -> stat3 ->
  AR -> w3_eff -> linear (transposed: out partitions = images) -> out.
"""
import sys
sys.path.insert(0, '/opt/trn_rl_repo')
import numpy as np

import concourse.bass as bass
import concourse.tile as tile
from concourse import bacc, mybir
from concourse import bass_utils

f32 = mybir.dt.float32
bf16 = mybir.dt.bfloat16
AF = mybir.ActivationFunctionType
ALU = mybir.AluOpType
AX = mybir.AxisListType

N_CORES = 8
NS = 2048 // N_CORES           # images per core

# conv1 tiling
C1_NCHUNK = 16
C1_CHUNKS = NS // C1_NCHUNK    # 16
C1_COLS = C1_NCHUNK * 12 * 24  # 4608 im2col cols per chunk
C1_N = 512                     # matmul moving cols (1 PSUM bank f32)
C1_MM = C1_COLS // C1_N        # 9
# conv2 tiling
C2_NCHUNK = 8
C2_CHUNKS = NS // C2_NCHUNK    # 32


def _cap(handle_ap, ap_list, offset):
    c = handle_ap.copy()
    c.ap = mybir.VecI64Pair(ap_list)
    c.offset = offset
    return c


def build_nc(n_cores=N_CORES):
    nc = bacc.Bacc(None, target_bir_lowering=False, num_devices=n_cores)
    rg = [list(range(n_cores))]
    inv_n = 1.0 / (NS * n_cores)

    x = nc.dram_tensor("x", [NS, 1, 28, 28], f32, kind="ExternalInput")
    P = {}
    for pre, wshape, cin in [("c1", [64, 1, 5, 5], 1), ("c2", [64, 64, 5, 5], 64)]:
        for s in ["w0", "wS2", "wG", "weta"]:
            P[f"{pre}_{s}"] = nc.dram_tensor(f"{pre}_{s}", wshape, f32, kind="ExternalInput")
        P[f"{pre}_wM"] = nc.dram_tensor(f"{pre}_wM", [cin], f32, kind="ExternalInput")
        for s in ["b0", "bS2", "bG", "beta"]:
            P[f"{pre}_{s}"] = nc.dram_tensor(f"{pre}_{s}", [64], f32, kind="ExternalInput")
    for s in ["w0", "wS2", "wG", "weta", "wM"]:
        P[f"l_{s}"] = nc.dram_tensor(f"l_{s}", [10, 1024], f32, kind="ExternalInput")
    for s in ["b0", "bS2", "bG", "beta"]:
        P[f"l_{s}"] = nc.dram_tensor(f"l_{s}", [10], f32, kind="ExternalInput")
    out = nc.dram_tensor("out", [NS, 10], f32, kind="ExternalOutput")

    with tile.TileContext(nc) as tc:
        with (
            tc.tile_pool(name="const", bufs=1) as cp,
            tc.tile_pool(name="work", bufs=2) as wp,
            tc.tile_pool(name="psum", bufs=1, space="PSUM") as pp,
            tc.tile_pool(name="dram", bufs=1, space="DRAM") as dp,
        ):
            # Packed f32 scratch [128, 420] (one 4KB slot). Column map:
            # 0 ones-col | 1:3 m_im | 3 s1 | 4 s1r | 5 wm1 | 6 m1 | 7 m1sq
            # 8 m1sqb64 | 9 acc2 | 10 s2r | 11 wm2 | 12 m2 | 13 m2sq
            # 14 pm1 | 15 c1_be | 16 c2_be | 24:152 ones-row (row 0)
            # 152:168 s3 | 168:184 s3r | 184:194 l_be row | 200:220 osb
            # 220:252 wtr | 256:320 wtrT | 320:420 w1 params (4x25)
            scal = cp.tile([128, 420], f32)
            ones_col = scal[:, 0:1]
            m_im = scal[:, 1:3]
            s1 = scal[0:1, 3:4]
            s1r = scal[0:1, 4:5]
            wm1 = scal[0:1, 5:6]
            m1 = scal[0:1, 6:7]
            m1sq = scal[0:1, 7:8]
            m1sqb = scal[0:64, 8:9]
            acc2 = scal[0:64, 9:10]
            s2r = scal[0:64, 10:11]
            wm2 = scal[0:64, 11:12]
            m2 = scal[0:64, 12:13]
            m2sq = scal[0:64, 13:14]
            pm1 = scal[0:64, 14:15]
            c1_be = scal[0:64, 15:16]
            c2_be = scal[0:64, 16:17]
            ones_row = scal[0:1, 24:152]          # [1,128]
            s3 = scal[0:64, 152:168]
            s3r = scal[0:64, 168:184]
            l_be = scal[0:1, 184:194]             # row layout [1,10]
            osb = scal[:, 200:220]                # [128, 2, 10] flat
            wtr = scal[0:64, 220:252]             # [64,32]
            wtrT = scal[0:32, 256:320]            # [32,64]
            w1p = scal[0:64, 320:420].rearrange("p (a b) -> p a b", a=4)  # [64,4,25]

            xbf = dp.tile([NS, 28, 28], bf16)
            imcol = dp.tile([30, NS * 12 * 24], bf16)

            # ---------------- A: load x, stat1, bf16 staging ----------------
            with tc.tile_pool(name="early", bufs=1) as ep:
                x_sb = ep.tile([128, 2, 784], f32)
                nc.sync.dma_start(
                    x_sb[:], x[:].rearrange("(i2 p) one h w -> p i2 (one h w)", p=128))
                nc.vector.tensor_reduce(m_im, x_sb[:], axis=AX.X, op=ALU.max,
                                        apply_absolute_value=True)
                nc.vector.memset(ones_col, 1.0)
                nc.vector.memset(ones_row, 1.0)
                ps1 = pp.tile([1, 2], f32, tag="small", bufs=2)
                nc.tensor.matmul(ps1[:], lhsT=ones_col, rhs=m_im, start=True, stop=True)
                nc.vector.tensor_reduce(s1, ps1[:], axis=AX.X, op=ALU.add)

                s1i = dp.tile([1, 1], f32)
                s1o = dp.tile([1, 1], f32)
                nc.sync.dma_start(s1i[:], s1)
                nc.gpsimd.collective_compute("AllReduce", ALU.add, replica_groups=rg,
                                             ins=[s1i.opt()], outs=[s1o.opt()])
                nc.sync.dma_start(s1r, s1o[:])

                nc.sync.dma_start(wm1, P["c1_wM"][:].unsqueeze(-1))
                nc.vector.tensor_scalar_mul(m1, s1r, inv_n)
                nc.vector.tensor_max(m1, m1, wm1)
                nc.vector.tensor_mul(m1sq, m1, m1)
                psb = pp.tile([64, 1], f32, tag="small", bufs=2)
                nc.tensor.matmul(psb[:], lhsT=ones_row[:, 0:64], rhs=m1sq,
                                 start=True, stop=True)
                nc.scalar.copy(m1sqb, psb[:])

                xbf_sb = ep.tile([128, 2, 784], bf16)
                nc.vector.tensor_copy(xbf_sb[:], x_sb[:])
                nc.sync.dma_start(
                    xbf[...].rearrange("(i2 p) h w -> p i2 (h w)", p=128), xbf_sb[:])

            # im2col staged in DRAM: strip (r,kj) <- x[n, 2*y2+r, x+kj]
            xd = xbf[...]
            for r in range(6):
                for kj in range(5):
                    src = _cap(xd, [[784, NS], [56, 12], [1, 24]], r * 28 + kj)
                    nc.sync.dma_start(imcol[r * 5 + kj:r * 5 + kj + 1, :], src)

            # ---------------- B: conv1 effective weights --------------------
            # w1p [64o, {w0,S2->rden,G->upd,eta}, 25tap]
            for i, s in enumerate(["w0", "wS2", "wG", "weta"]):
                nc.sync.dma_start(w1p[:, i, :],
                                  P[f"c1_{s}"][:].rearrange("o one ki kj -> o (one ki kj)"))
            nc.scalar.activation(w1p[:, 1, :], w1p[:, 1, :], AF.Sqrt, bias=m1sqb)
            nc.vector.reciprocal(w1p[:, 1, :], w1p[:, 1, :])
            nc.vector.tensor_mul(w1p[:, 2, :], w1p[:, 2, :], w1p[:, 1, :])
            nc.vector.tensor_scalar(w1p[:, 2, :], w1p[:, 2, :], 1.0, -1.0,
                                    op0=ALU.min, op1=ALU.max)
            nc.vector.tensor_mul(w1p[:, 2, :], w1p[:, 2, :], w1p[:, 1, :])
            nc.vector.tensor_mul(w1p[:, 2, :], w1p[:, 2, :], w1p[:, 3, :])
            nc.vector.tensor_scalar_mul(w1p[:, 2, :], w1p[:, 2, :], 0.5)
            nc.vector.tensor_add(wtr[:, 0:25], w1p[:, 0, :], w1p[:, 2, :])
            # transpose [64o, 25tap] -> [25tap, 64o] via DVE 32x32 blocks
            nc.vector.memset(wtr[:, 25:32], 0.0)
            nc.vector.transpose(wtrT[:, 0:32], wtr[0:32, :])
            nc.vector.transpose(wtrT[:, 32:64], wtr[32:64, :])
            # lhsT1 [30,128] bf16: row p=(r*5+kj); col m=(dy*64+o); w1e[(r-dy),kj]
            lhsT1 = cp.tile([30, 128], bf16)
            wtb = cp.tile([32, 64], bf16)
            nc.vector.memset(lhsT1[:], 0.0)
            nc.vector.tensor_copy(wtb[:], wtrT[:])
            nc.vector.tensor_copy(lhsT1[0:25, 0:64], wtb[0:25, :])
            # partition base 5 is not addressable by compute engines; DMA it
            nc.sync.dma_start(lhsT1[5:30, 64:128], wtb[0:25, :])

            # column-layout effective bias for c1/c2: bp [64, 6]
            def bias_eff_col(pre, be_dst):
                bp = wp.tile([64, 6], f32, tag="biasp", bufs=1)
                for i, s in enumerate(["b0", "bS2", "bG", "beta"]):
                    nc.sync.dma_start(bp[:, i:i + 1], P[f"{pre}_{s}"][:].unsqueeze(-1))
                nc.scalar.activation(bp[:, 1:2], bp[:, 1:2], AF.Sqrt, bias=1.0)
                nc.vector.reciprocal(bp[:, 1:2], bp[:, 1:2])
                nc.vector.tensor_mul(bp[:, 2:3], bp[:, 2:3], bp[:, 1:2])
                nc.vector.tensor_scalar(bp[:, 2:3], bp[:, 2:3], 1.0, -1.0,
                                        op0=ALU.min, op1=ALU.max)
                nc.vector.tensor_mul(bp[:, 2:3], bp[:, 2:3], bp[:, 1:2])
                nc.vector.tensor_mul(bp[:, 2:3], bp[:, 2:3], bp[:, 3:4])
                nc.vector.tensor_scalar_mul(bp[:, 2:3], bp[:, 2:3], 0.5)
                nc.vector.tensor_add(be_dst, bp[:, 0:1], bp[:, 2:3])

            bias_eff_col("c1", c1_be)

            # ---------------- D: conv1 + pool1 + stat2 ----------------------
            h1 = cp.tile([128, NS, 12, 12], bf16)  # upper: h1; lower: x-shift +1
            h1u = h1[0:64, :, :, :].rearrange("p a b c -> p (a b c)")
            h1l = h1[64:128, :, :, :].rearrange("p a b c -> p (a b c)")
            nc.vector.memset(acc2, 0.0)
            HALF = C1_COLS // 2                    # pooled cols per chunk
            for c in range(C1_CHUNKS):
                n0 = c * C1_NCHUNK
                ic = wp.tile([30, C1_COLS], bf16, tag="ic", bufs=2)
                nc.sync.dma_start(ic[:], imcol[:, c * C1_COLS:(c + 1) * C1_COLS])
                for j in range(C1_MM):
                    ps = pp.tile([128, C1_N], f32, tag="mm", bufs=4)
                    nc.tensor.matmul(ps[:], lhsT=lhsT1[:],
                                     rhs=ic[:, j * C1_N:(j + 1) * C1_N],
                                     start=True, stop=True)
                    pw = wp.tile([128, 512], f32, tag="pw", bufs=2)
                    q1 = pw[:, 0:256]
                    t2 = pw[0:64, 256:512]
                    nc.vector.tensor_max(q1, ps[:, 0::2], ps[:, 1::2])
                    nc.vector.tensor_max(t2, q1[0:64, :], q1[64:128, :])
                    base = c * HALF + j * (C1_N // 2)
                    nc.scalar.activation(h1u[:, base:base + C1_N // 2], t2, AF.Relu,
                                         bias=c1_be)
                base = c * HALF
                if c == 0:
                    nc.vector.tensor_copy(h1l[:, 0:HALF - 1], h1u[:, 1:HALF])
                else:
                    nc.vector.tensor_copy(h1l[:, base - 1:base + HALF - 1],
                                          h1u[:, base:base + HALF])
                pm = wp.tile([64, C1_NCHUNK], f32, tag="pm", bufs=2)
                nc.vector.tensor_reduce(pm[:], h1[0:64, n0:n0 + C1_NCHUNK, :, :],
                                        axis=AX.XY, op=ALU.max)
                nc.vector.tensor_reduce(pm1, pm[:], axis=AX.X, op=ALU.add)
                nc.vector.tensor_add(acc2, acc2, pm1)

            # ---------------- F: AR2, M2 ------------------------------------
            s2i = dp.tile([64, 1], f32)
            s2o = dp.tile([64, 1], f32)
            nc.sync.dma_start(s2i[:], acc2)
            nc.gpsimd.collective_compute("AllReduce", ALU.add, replica_groups=rg,
                                         ins=[s2i.opt()], outs=[s2o.opt()])
            nc.sync.dma_start(s2r, s2o[:])
            nc.sync.dma_start(wm2, P["c2_wM"][:].unsqueeze(-1))
            nc.vector.tensor_scalar_mul(m2, s2r, inv_n)
            nc.vector.tensor_max(m2, m2, wm2)
            nc.vector.tensor_mul(m2sq, m2, m2)

            # ---------------- G: conv2 effective weights (2 tap halves) -----
            # layout [cin, o, tap] so DMA last dim (tap) is contiguous
            w2s = cp.tile([128, 64, 25], bf16)
            for t0, tn in [(0, 13), (13, 12)]:
                w2p = wp.tile([64, 4, 64, 13], f32, tag="w2p", bufs=1)
                v = w2p[:, :, :, 0:tn]
                for i, s in enumerate(["w0", "wS2", "wG", "weta"]):
                    src = P[f"c2_{s}"][:].rearrange("o c ki kj -> c o (ki kj)")
                    nc.sync.dma_start(w2p[:, i, :, 0:tn], src[:, :, t0:t0 + tn])
                nc.scalar.activation(v[:, 1], v[:, 1], AF.Sqrt, bias=m2sq)
                nc.vector.reciprocal(v[:, 1], v[:, 1])
                nc.vector.tensor_mul(v[:, 2], v[:, 2], v[:, 1])
                nc.vector.tensor_scalar(v[:, 2], v[:, 2], 1.0, -1.0,
                                        op0=ALU.min, op1=ALU.max)
                nc.vector.tensor_mul(v[:, 2], v[:, 2], v[:, 1])
                nc.vector.tensor_mul(v[:, 2], v[:, 2], v[:, 3])
                nc.vector.tensor_scalar_mul(v[:, 2], v[:, 2], 0.5)
                nc.vector.tensor_add(w2s[0:64, :, t0:t0 + tn], v[:, 0], v[:, 2])
            nc.vector.tensor_copy(w2s[64:128, :, 0:24], w2s[0:64, :, 1:25])
            nc.vector.memset(w2s[64:128, :, 24:25], 0.0)
            bias_eff_col("c2", c2_be)

            # ---------------- H: conv2 + pool2 ------------------------------
            h2 = cp.tile([64, NS, 4, 4], bf16)
            for c in range(C2_CHUNKS):
                n0 = c * C2_NCHUNK
                ps2 = pp.tile([64, C2_NCHUNK, 8, 8], f32, tag="mm", bufs=4)
                first = True
                for ki in range(5):
                    for kj, paired in [(0, True), (2, True), (4, False)]:
                        kk = 128 if paired else 64
                        nc.tensor.matmul(
                            ps2[:],
                            lhsT=w2s[0:kk, :, ki * 5 + kj],
                            rhs=h1[0:kk, n0:n0 + C2_NCHUNK, ki:ki + 8, kj:kj + 8],
                            start=first, stop=(ki == 4 and kj == 4))
                        first = False
                pwb = wp.tile([64, 384], f32, tag="pw", bufs=2)
                q1b = pwb[:, 0:256].rearrange("p (a b c) -> p a b c", a=8, b=8)
                q2b = pwb[:, 256:384].rearrange("p (a b c) -> p a b c", a=8, b=4)
                nc.vector.tensor_max(q1b, ps2[:, :, :, 0::2], ps2[:, :, :, 1::2])
                nc.vector.tensor_max(q2b, q1b[:, :, 0::2, :], q1b[:, :, 1::2, :])
                nc.scalar.activation(h2[:, n0:n0 + C2_NCHUNK, :, :],
                                     q2b, AF.Relu, bias=c2_be)

            # ---------------- I: stat3 + AR3 --------------------------------
            nc.vector.tensor_reduce(s3, h2[:, :, :, :].rearrange("p n y x -> p y x n"),
                                    axis=AX.X, op=ALU.add)
            s3i = dp.tile([64, 16], f32)
            s3o = dp.tile([64, 16], f32)
            nc.sync.dma_start(s3i[:], s3)
            nc.gpsimd.collective_compute("AllReduce", ALU.add, replica_groups=rg,
                                         ins=[s3i.opt()], outs=[s3o.opt()])
            nc.sync.dma_start(s3r, s3o[:])
            nc.vector.tensor_scalar_mul(s3r, s3r, inv_n)

            # ---------------- J: linear effective weights -------------------
            # pk3 [64, 6, 10, 16]: w0 | S2->rden | G->upd | eta | wM->M3^2 | mask
            pk3 = cp.tile([64, 6, 10, 16], f32)
            for i, s in enumerate(["w0", "wS2", "wG", "weta", "wM"]):
                nc.sync.dma_start(pk3[:, i, :, :], P[f"l_{s}"][:].rearrange(
                    "o (c y x) -> c o (y x)", c=64, y=4, x=4))
            nc.vector.tensor_scalar(pk3[:, 5], pk3[:, 1], 0.0, None, op0=ALU.not_equal)
            nc.vector.tensor_max(pk3[:, 4], pk3[:, 4],
                                 s3r.unsqueeze(1).broadcast_to([64, 10, 16]))
            nc.vector.tensor_mul(pk3[:, 4], pk3[:, 4], pk3[:, 4])
            nc.vector.tensor_add(pk3[:, 1], pk3[:, 1], pk3[:, 4])
            nc.scalar.activation(pk3[:, 1], pk3[:, 1], AF.Sqrt)
            nc.vector.reciprocal(pk3[:, 1], pk3[:, 1])
            nc.vector.tensor_mul(pk3[:, 2], pk3[:, 2], pk3[:, 1])
            nc.vector.tensor_scalar(pk3[:, 2], pk3[:, 2], 1.0, -1.0,
                                    op0=ALU.min, op1=ALU.max)
            nc.vector.tensor_mul(pk3[:, 2], pk3[:, 2], pk3[:, 1])
            nc.vector.tensor_mul(pk3[:, 2], pk3[:, 2], pk3[:, 3])
            nc.vector.tensor_scalar_mul(pk3[:, 2], pk3[:, 2], 0.5)
            nc.vector.tensor_mul(pk3[:, 2], pk3[:, 2], pk3[:, 5])
            w3b = cp.tile([64, 10, 16], bf16)
            nc.vector.tensor_add(w3b[:], pk3[:, 0], pk3[:, 2])

            # l bias in row layout [1, 10]
            lp = wp.tile([1, 50], f32, tag="lrow", bufs=1)
            for i, s in enumerate(["b0", "bS2", "bG", "beta"]):
                nc.sync.dma_start(lp[:, i * 10:(i + 1) * 10], P[f"l_{s}"][:].unsqueeze(0))
            nc.vector.tensor_scalar(lp[:, 40:50], lp[:, 10:20], 0.0, None,
                                    op0=ALU.not_equal)
            nc.scalar.activation(lp[:, 10:20], lp[:, 10:20], AF.Sqrt, bias=1.0)
            nc.vector.reciprocal(lp[:, 10:20], lp[:, 10:20])
            nc.vector.tensor_mul(lp[:, 20:30], lp[:, 20:30], lp[:, 10:20])
            nc.vector.tensor_scalar(lp[:, 20:30], lp[:, 20:30], 1.0, -1.0,
                                    op0=ALU.min, op1=ALU.max)
            nc.vector.tensor_mul(lp[:, 20:30], lp[:, 20:30], lp[:, 10:20])
            nc.vector.tensor_mul(lp[:, 20:30], lp[:, 20:30], lp[:, 30:40])
            nc.vector.tensor_scalar_mul(lp[:, 20:30], lp[:, 20:30], 0.5)
            nc.vector.tensor_mul(lp[:, 20:30], lp[:, 20:30], lp[:, 40:50])
            nc.vector.tensor_add(l_be, lp[:, 0:10], lp[:, 20:30])

            # ---------------- K: linear + output ----------------------------
            # out.T blocks: psum [128 imgs, 10]; lhsT = h2 slice; rhs = w3b
            h2v = h2[:, :, :, :].rearrange("p n y x -> p (y x) n")
            for h in range(2):
                pl = pp.tile([128, 10], f32, tag="small", bufs=2)
                for yx in range(16):
                    nc.tensor.matmul(pl[:], lhsT=h2v[:, yx, h * 128:(h + 1) * 128],
                                     rhs=w3b[:, :, yx], start=(yx == 0), stop=False)
                nc.tensor.matmul(pl[:], lhsT=ones_row, rhs=l_be, start=False, stop=True)
                nc.scalar.copy(osb[:, h * 10:(h + 1) * 10], pl[:])
            nc.sync.dma_start(
                out[:].rearrange("(h p) o -> p h o", p=128),
                osb[:].rearrange("p (h o) -> p h o", h=2))

    nc.finalize()
    return nc


_CACHED = {}


def _get_nc(n_cores=N_CORES):
    if n_cores not in _CACHED:
        _CACHED[n_cores] = build_nc(n_cores)
    return _CACHED[n_cores]


def kernel(**inputs):
    nc = _get_nc(N_CORES)
    x = np.asarray(inputs["x"], dtype=np.float32)
    params = {k: np.asarray(v, dtype=np.float32) for k, v in inputs.items() if k != "x"}
    in_maps = []
    for c in range(N_CORES):
        m = {"x": np.ascontiguousarray(x[c * NS:(c + 1) * NS])}
        m.update(params)
        in_maps.append(m)
    res = bass_utils.run_bass_kernel_spmd(nc, in_maps, core_ids=list(range(N_CORES)))
    return np.concatenate([res.results[c]["out"] for c in range(N_CORES)], axis=0)
